# revision 36
# baseline (speedup 1.0000x reference)
"""HMM loss kernel for Trainium2 (8 NeuronCores, vocab-parallel).

Problem shapes (hardcoded): B,T,K,LS = 4,8,4,4; PH=B*T*K=128, TL=32,
H=512, V=32000, NS=128.

Only tokens inside the inclusive span [tgt_idx[p,0], tgt_idx[p,1]] matter
(~1500 of 4096), and log_softmax is only consumed via
psk = logit[target] - log(sum_v exp(logit_v)).  The vocab axis is sharded
over the 8 cores (4000 columns each).

Per-core device program (all engines balanced near their cost-model rates):
  - PE: fp8 DoubleRow matmul [128tok, 512] x [512, 4000] per 128-token chunk
    into PSUM, preceded by dummy bf16 warm-up matmuls that keep the PE
    p-state ramped while the input DMAs stream in.
  - ACT (the only engine with exp): exponentiates columns [0, 2048) of each
    chunk in-place with the fp8 scale folded in, accumulating the row sum
    (accum_out), one instruction per 1024-column stripe.
  - DVE: casts columns [2048, 4000) f32->bf16 into SBUF staging.
  - DMA: ships the bf16 logit stripes to DRAM; the host exponentiates and
    sums those columns in float64 (identical math, just partitioned).
PSUM budget: 2 ping-pong buffers x 2 banks for the ACT stream + the same
for the DVE stream = all 8 banks.

Host side: target-logit dots (einsum over ~1500x512), combining the
device/host sum-exp partials, and the tiny T=8/K=4 backward scan.
"""

import math
from contextlib import ExitStack

import ml_dtypes
import numpy as np

B, T, K, LS = 4, 8, 4, 4
PH, TL, H, V, NS = B * T * K, 32, 512, 32000, 128
NCORES = 8
VS = V // NCORES  # vocab shard per core (4000)
HC = H // 128  # contraction subtiles
QW = 512  # w-tile width (one DMA each)
# Columns [0, A_END) are exp'd on device by ACT (two stripes in 2-bank
# ping-pong PSUM buffers); columns [A_END, VS) are relayed by DVE as
# bf16 to the host through 512-col (1-bank) PSUM tiles on a 4-deep
# rotation (a 2-deep rotation is unstable: the DVE-copy + sem + refill
# cycle exceeds the sub-stripe period and the PE stalls cascade).
A_END = 2048
DW = VS - A_END  # host-summed columns per chunk (shipped as bf16)
XSCALE = 16.0  # fp8 pre-scales keep W (sigma~0.02) out of e4m3 subnormals
WSCALE = 256.0
EXP_SCALE = 1.0 / (XSCALE * WSCALE)
WARM = 26  # PE warm-up matmuls (~107ns each at mid clock)


def _split_sync_waits(nc, maxw=1):
    """This container's walrus rejects instructions carrying more than a
    couple of sync-wait commands, while Tile freely attaches one wait per
    dependency.  Hoist excess waits onto standalone EventSemaphore
    instructions inserted just before the owner on the same engine queue."""
    import concourse.mybir as mybir

    ctr = 0
    for fn in nc.m.functions:
        for bb in fn.blocks:
            out = []
            changed = False
            for inst in bb.instructions:
                si = getattr(inst, "sync_info", None)
                waits = list(si.on_wait) if si is not None and si.on_wait else []
                if len(waits) > maxw:
                    changed = True
                    extra, keep = waits[:-maxw], waits[-maxw:]
                    for i in range(0, len(extra), maxw):
                        ctr += 1
                        out.append(
                            mybir.InstEventSemaphore(
                                name=f"W-split-{ctr}",
                                engine=inst.engine,
                                ins=[],
                                outs=[],
                                sync_info=mybir.SyncInfo(
                                    on_wait=extra[i : i + maxw], on_update=[]
                                ),
                            )
                        )
                    inst.sync_info = mybir.SyncInfo(
                        on_wait=keep, on_update=list(si.on_update or [])
                    )
                out.append(inst)
            if changed:
                bb.instructions = out


_BUILD_CACHE = {}


def _build(n_chunks, ntc=0, with_bias=False, repeat=1, warm=WARM, worder=None):
    """Build the per-core bass program.  `ntc` is unused (kept for the
    test harness call signature); `repeat` re-emits the whole body for
    marginal-cost timing; `worder` overrides the w-tile DMA issue order."""
    key = (n_chunks, with_bias, repeat, warm, tuple(worder) if worder else None)
    if key in _BUILD_CACHE:
        return _BUILD_CACHE[key]

    import concourse.bass as bass
    import concourse.mybir as mybir
    import concourse.tile as tile

    n_pad = n_chunks * 128
    n_xt = math.ceil(n_pad / 512)  # xt DMA granularity: 4 chunks
    f8 = mybir.dt.float8e4
    bf16 = mybir.dt.bfloat16
    f32 = mybir.dt.float32
    nslots = 2 * n_chunks

    nc = bass.Bass()
    # [partition, k-subtile, col] layout: element (p, s, c) = row s*128+p
    xt_d = nc.dram_tensor("xt", [128, HC, n_pad], f8, kind="ExternalInput")
    w_d = nc.dram_tensor("w", [128, HC, VS], f8, kind="ExternalInput")
    if with_bias:
        bb_d = nc.dram_tensor("bb", [1, VS], bf16, kind="ExternalInput")
    se_d = nc.dram_tensor("se", [128, nslots], f32, kind="ExternalOutput")
    # last chunk ships A_END//2 extra host columns (its ACT share is halved
    # so ACT's final accum — which gates the se DMA chain — lands earlier)
    lg_d = nc.dram_tensor(
        "lg", [n_chunks, 128, DW + A_END // 2], bf16, kind="ExternalOutput"
    )

    with tile.TileContext(nc) as tc, ExitStack() as ctx:
        consts = ctx.enter_context(tc.tile_pool(name="consts", bufs=1))
        psA = ctx.enter_context(tc.tile_pool(name="psA", bufs=2, space="PSUM"))
        psD = ctx.enter_context(tc.tile_pool(name="psD", bufs=4, space="PSUM"))
        stage = ctx.enter_context(tc.tile_pool(name="stage", bufs=3))
        outp = ctx.enter_context(tc.tile_pool(name="outp", bufs=1))

        for _rep in range(repeat):
            # ---- input DMAs, ordered so chunk0's dependencies land first
            nw = math.ceil(VS / QW)
            w_sb = []
            for q in range(nw):
                qw = min(QW, VS - q * QW)
                w_sb.append(
                    consts.tile([128, HC, qw], f8, tag=f"wq{q}", name=f"wq{q}")
                )
            xt_sb = []
            for t in range(n_xt):
                tw = min(512, n_pad - 512 * t)
                xt_sb.append(
                    consts.tile([128, HC, tw], f8, tag=f"xt{t}", name=f"xt{t}")
                )

            def dma_w(q):
                nc.sync.dma_start(
                    out=w_sb[q],
                    in_=w_d[:, :, q * QW : q * QW + w_sb[q].shape[2]],
                )

            def dma_xt(t):
                nc.sync.dma_start(
                    out=xt_sb[t],
                    in_=xt_d[:, :, 512 * t : 512 * t + xt_sb[t].shape[2]],
                )

            # DVE is the pacing engine (relay budget ~2533/chunk vs ACT's
            # 2368), so its inputs land first: xt0, then the D-region
            # w-tiles, then ACT's w-tiles (whose first exp can start ~3us
            # later without moving the finish line), then later xt chunks.
            wo = worder or (list(range(A_END // QW, nw))
                            + list(range(A_END // QW)))
            dma_xt(0)
            for q in wo:
                dma_w(q)
            for t in range(1, n_xt):
                dma_xt(t)
            if with_bias:
                ones_sb = consts.tile([1, 128], bf16, tag="ones")
                nc.vector.memset(ones_sb, 1.0)
                b_sb = consts.tile([1, VS], bf16, tag="bias")
                nc.sync.dma_start(out=b_sb, in_=bb_d[0:1, :])

            se_all = outp.tile([128, nslots], f32, tag="se")
            # the last chunk only writes one accum column; zero the rest so
            # the host can sum columns uniformly
            nc.vector.memset(se_all, 0.0)

            # ---- PE warm-up: keep the tensor engine's p-state ramping while
            # the input DMAs stream.  Writes are overwritten by the first
            # real matmul (start=True); PE executes in program order.
            dummy = consts.tile([128, 128], bf16, tag="dummy")
            nc.vector.memset(dummy, 0.0)
            pw = psA.tile([128, A_END // 2], f32, name="psa")
            for _ in range(warm):
                nc.tensor.matmul(pw[:, 0:128], lhsT=dummy, rhs=dummy)

            def mm_stripe(ps, lo, hi, c):
                """Matmuls filling ps[:, 0:hi-lo] with logits for vocab
                columns [lo, hi) of 128-token chunk c."""
                xs = xt_sb[c // 4]
                coff = 128 * (c % 4)
                cuts = [lo]
                while cuts[-1] < hi:
                    cuts.append(min((cuts[-1] // 512 + 1) * 512, hi))
                for j in range(len(cuts) - 1):
                    a, b2 = cuts[j], cuts[j + 1]
                    q, qo = a // QW, a % QW
                    pslice = ps[:, a - lo : b2 - lo]
                    for s in range(0, HC, 2):
                        nc.tensor.matmul(
                            pslice,
                            lhsT=xs[:, s : s + 2, coff : coff + 128],
                            rhs=w_sb[q][:, s : s + 2, qo : qo + (b2 - a)],
                            start=(s == 0),
                            stop=(s == HC - 2) and not with_bias,
                            perf_mode=mybir.MatmulPerfMode.DoubleRow,
                        )
                    if with_bias:
                        nc.tensor.matmul(
                            pslice,
                            lhsT=ones_sb,
                            rhs=b_sb[:, a:b2],
                            start=False,
                            stop=True,
                        )

            AS = A_END // 2  # ACT stripe width

            def emit_A(c, a_hi):
                for i in range(a_hi // AS):  # ACT stripes cover [0, a_hi)
                    ps = psA.tile([128, AS], f32, name="psa")
                    mm_stripe(ps, i * AS, (i + 1) * AS, c)
                    nc.scalar.activation(
                        out=ps,
                        in_=ps,
                        func=mybir.ActivationFunctionType.Exp,
                        scale=EXP_SCALE,
                        accum_out=se_all[:, 2 * c + i : 2 * c + i + 1],
                    )

            def emit_D(c, a_hi, last):
                # host sub-stripes: columns [a_hi, VS) in 512-col pieces
                stw = VS - a_hi
                st = stage.tile([128, stw], bf16, name=f"st{a_hi}")
                lo = a_hi
                while lo < VS:
                    hi = min(lo + 512, VS)
                    w0 = hi - lo
                    ps = psD.tile([128, 512], f32, name="psd")
                    mm_stripe(ps, lo, hi, c)
                    nc.vector.tensor_copy(
                        st[:, lo - a_hi : hi - a_hi], ps[:, :w0]
                    )
                    if last:  # split the final chunk's DMA, shortens the tail
                        nc.sync.dma_start(
                            out=lg_d[c][:, lo - a_hi : hi - a_hi],
                            in_=st[:, lo - a_hi : hi - a_hi],
                        )
                    lo = hi
                if not last:
                    nc.sync.dma_start(out=lg_d[c][:, :stw], in_=st)

            for c in range(n_chunks):
                last = c == n_chunks - 1
                if last:
                    # final chunk: halve ACT's share (its last accum gates
                    # the se DMA chain; DVE finishes earlier) and emit the
                    # ACT stripe first
                    emit_A(c, AS)
                    emit_D(c, AS, last)
                else:
                    # D sub-stripes first: keeps the PE feeding the pacing
                    # DVE as early as possible each chunk
                    emit_D(c, A_END, last)
                    emit_A(c, A_END)

            nc.sync.dma_start(out=se_d[:, :], in_=se_all)

    _split_sync_waits(nc)
    _BUILD_CACHE[key] = nc
    return nc


def _prep_inputs(output, W, b, target, tgt_idx, fp8=True):
    """Host-side sharding/layout prep. Returns (in_maps, meta)."""
    x = np.asarray(output, np.float32).reshape(PH * TL, H)
    tgt = np.asarray(target, np.int32).reshape(-1)
    ti = np.asarray(tgt_idx, np.int32)
    bv = np.asarray(b, np.float32).reshape(-1)
    with_bias = bool(np.any(bv != 0.0))

    pos = np.arange(TL)
    span = (pos[None, :] >= ti[:, :1]) & (pos[None, :] <= ti[:, 1:2])
    act = np.flatnonzero(span.reshape(-1))
    n_act = int(act.size)
    n_chunks = max(1, math.ceil(n_act / 128))
    n_pad = n_chunks * 128
    act_pad = np.zeros(n_pad, np.int64)
    act_pad[:n_act] = act

    Wf = np.asarray(W, np.float32)
    xa = x[act_pad]  # [n_pad, H] f32

    x_m = (xa * XSCALE).astype(ml_dtypes.float8_e4m3)
    w_m = (Wf * WSCALE).astype(ml_dtypes.float8_e4m3)

    # [H, n_pad] -> [HC,128,n_pad] -> [128,HC,n_pad] (partition, k-subtile, col)
    xt = np.ascontiguousarray(x_m.T.reshape(HC, 128, n_pad).transpose(1, 0, 2))

    in_maps = []
    for i in range(NCORES):
        wsh = np.ascontiguousarray(
            w_m[:, i * VS : (i + 1) * VS].reshape(HC, 128, VS).transpose(1, 0, 2)
        )
        m = {"xt": xt, "w": wsh}
        if with_bias:
            m["bb"] = (bv[i * VS : (i + 1) * VS] * (XSCALE * WSCALE)).astype(
                ml_dtypes.bfloat16
            ).reshape(1, VS)
        in_maps.append(m)

    meta = dict(
        act=act, act_pad=act_pad, n_act=n_act, n_chunks=n_chunks, n_pad=n_pad,
        ntc=0, tgt=tgt, with_bias=with_bias, bv=bv, xa=xa, Wf=Wf,
    )
    return in_maps, meta


def _combine(results, meta):
    """Combine device sum-exp partials with host-side exp of the shipped
    bf16 logit stripes; return psk[PH, TL] (float64)."""
    n_act, n_pad, n_chunks = meta["n_act"], meta["n_pad"], meta["n_chunks"]
    total = np.zeros(n_pad, np.float64)
    for r in results:
        se = r["se"].astype(np.float64)  # [128, 2*n_chunks]
        dev = se[:, 0::2] + se[:, 1::2]  # [128, n_chunks]
        total += dev.T.reshape(-1)
        lg = r["lg"]  # [n_chunks, 128, DW + A_END//2] bf16 (raw scaled logits)
        for c in range(n_chunks):
            w0 = DW + A_END // 2 if c == n_chunks - 1 else DW
            e = np.exp(lg[c][:, :w0].astype(np.float32) * np.float32(EXP_SCALE))
            total[c * 128 : (c + 1) * 128] += e.sum(axis=1, dtype=np.float64)

    # target logits on the host (exact f64 dots; ~1500 x 512)
    xa = meta["xa"][:n_act].astype(np.float64)
    wt = meta["Wf"][:, meta["tgt"][meta["act"]]].T.astype(np.float64)
    tl = (xa * wt).sum(axis=1)
    if meta["with_bias"]:
        tl = tl + meta["bv"][meta["tgt"][meta["act"]]]

    logz = np.log(total[:n_act])
    psk = np.zeros(PH * TL, np.float64)
    psk[meta["act"]] = tl - logz
    return psk.reshape(PH, TL)


def _hmm_tail(psk, tgt_idx, states, init_logps, trans_logps, ext_logps, hsmm_sid):
    """Direct numpy port of the reference below the log-softmax."""
    ti = np.asarray(tgt_idx, np.int32)
    st4 = np.asarray(states, np.int64)
    init_logps = np.asarray(init_logps, np.float64)
    trans_logps = np.asarray(trans_logps, np.float64)
    ext_logps = np.asarray(ext_logps, np.float64)
    sid = int(np.asarray(hsmm_sid))

    pos = np.arange(TL)
    span = (pos[None, :] >= ti[:, :1]) & (pos[None, :] <= ti[:, 1:2])
    fwd_obs = np.where(span, psk, 0.0).sum(axis=1)  # [PH]

    st = st4.reshape(PH, LS)
    chain = trans_logps[st[:, :-1], st[:, 1:]].sum(axis=1)  # [PH]
    init_pmt = (init_logps[st[:, 0]] + chain).reshape(B, T, K)
    pmt = chain.reshape(B, T, K)
    obs = fwd_obs.reshape(B, T, K)
    z = np.where((np.arange(T) == 0)[None, :, None], init_pmt, pmt)
    s_first = st4[..., 0]  # [B,T,K]
    s_last = st4[..., -1]
    ov = np.any(
        st4[:, :-1, :, None, :, None] == st4[:, 1:, None, :, None, :], axis=(-1, -2)
    )  # [B,T-1,K,K]

    def lse2(x):  # logsumexp over last axis, -inf safe
        m = np.max(x, axis=-1, keepdims=True)
        ms = np.where(np.isfinite(m), m, 0.0)
        with np.errstate(divide="ignore"):
            return np.log(np.exp(x - ms).sum(axis=-1)) + ms[..., 0]

    beta = np.zeros((B, K), np.float64)
    for t in range(T - 2, -1, -1):
        sl = s_last[:, t]
        sf = s_first[:, t + 1]
        tr = (
            trans_logps[sl[:, :, None], sf[:, None, :]]
            + ext_logps[sl[:, :, None], sf[:, None, :]]
        )
        score = (
            beta[:, None, :]
            + obs[:, t + 1][:, None, :]
            + z[:, t + 1][:, None, :]
            + z[:, t][:, :, None]
            + tr
        )
        if K > 1:
            score = np.where(ov[:, t], -np.inf, score)
        beta = lse2(score)

    score0 = beta + obs[:, 0] + z[:, 0] + ext_logps[sid, s_first[:, 0]]
    log_marg = lse2(score0)
    return -np.sum(log_marg)


def kernel(output, W, b, target, tgt_idx, states, init_logps, trans_logps,
           ext_logps, hsmm_sid):
    from concourse.bass_utils import run_bass_kernel_spmd

    in_maps, meta = _prep_inputs(output, W, b, target, tgt_idx)
    nc = _build(meta["n_chunks"], meta["ntc"], meta["with_bias"])
    last_err = None
    for _attempt in range(3):
        try:
            res = run_bass_kernel_spmd(nc, in_maps, core_ids=list(range(NCORES)))
            break
        except Exception as e:  # rare transient device-unrecoverable flakes
            last_err = e
            import time as _time

            _time.sleep(2.0)
    else:
        raise last_err
    psk = _combine(res.results, meta)
    loss = _hmm_tail(psk, tgt_idx, states, init_logps, trans_logps, ext_logps, hsmm_sid)
    return np.float32(loss)


# revision 53
# speedup vs baseline: 1.0046x; 1.0046x over previous
"""HMM loss kernel for Trainium2 (8 NeuronCores, vocab-parallel).

Problem shapes (hardcoded): B,T,K,LS = 4,8,4,4; PH=B*T*K=128, TL=32,
H=512, V=32000, NS=128.

Only tokens inside the inclusive span [tgt_idx[p,0], tgt_idx[p,1]] matter
(~1500 of 4096), and log_softmax is only consumed via
psk = logit[target] - log(sum_v exp(logit_v)).  The vocab axis is sharded
over the 8 cores (4000 columns each).

Per-core device program (all engines balanced near their cost-model rates):
  - PE: fp8 DoubleRow matmul [128tok, 512] x [512, 4000] per 128-token chunk
    into PSUM, preceded by dummy bf16 warm-up matmuls that keep the PE
    p-state ramped while the input DMAs stream in.
  - ACT (the only engine with exp): exponentiates columns [0, 2048) of each
    chunk in-place with the fp8 scale folded in, accumulating the row sum
    (accum_out), one instruction per 1024-column stripe.
  - DVE: casts columns [2048, 4000) f32->bf16 into SBUF staging.
  - DMA: ships the bf16 logit stripes to DRAM; the host exponentiates and
    sums those columns in float64 (identical math, just partitioned).
PSUM budget: 2 ping-pong buffers x 2 banks for the ACT stream + the same
for the DVE stream = all 8 banks.

Host side: target-logit dots (einsum over ~1500x512), combining the
device/host sum-exp partials, and the tiny T=8/K=4 backward scan.
"""

import math
from contextlib import ExitStack

import ml_dtypes
import numpy as np

B, T, K, LS = 4, 8, 4, 4
PH, TL, H, V, NS = B * T * K, 32, 512, 32000, 128
NCORES = 8
VS = V // NCORES  # vocab shard per core (4000)
HC = H // 128  # contraction subtiles
QW = 512  # w-tile width (one DMA each)
# Columns [0, A_END) are exp'd on device by ACT (two stripes in 2-bank
# ping-pong PSUM buffers); columns [A_END, VS) are relayed by DVE as
# bf16 to the host through 512-col (1-bank) PSUM tiles on a 4-deep
# rotation (a 2-deep rotation is unstable: the DVE-copy + sem + refill
# cycle exceeds the sub-stripe period and the PE stalls cascade).
A_END = 2048
DW = VS - A_END  # host-summed columns per chunk (shipped as bf16)
XSCALE = 16.0  # fp8 pre-scales keep W (sigma~0.02) out of e4m3 subnormals
WSCALE = 256.0
EXP_SCALE = 1.0 / (XSCALE * WSCALE)
WARM = 26  # PE warm-up matmuls (~107ns each at mid clock)


def _split_sync_waits(nc, maxw=1):
    """This container's walrus rejects instructions carrying more than a
    couple of sync-wait commands, while Tile freely attaches one wait per
    dependency.  Hoist excess waits onto standalone EventSemaphore
    instructions inserted just before the owner on the same engine queue."""
    import concourse.mybir as mybir

    ctr = 0
    for fn in nc.m.functions:
        for bb in fn.blocks:
            out = []
            changed = False
            for inst in bb.instructions:
                si = getattr(inst, "sync_info", None)
                waits = list(si.on_wait) if si is not None and si.on_wait else []
                if len(waits) > maxw:
                    changed = True
                    extra, keep = waits[:-maxw], waits[-maxw:]
                    for i in range(0, len(extra), maxw):
                        ctr += 1
                        out.append(
                            mybir.InstEventSemaphore(
                                name=f"W-split-{ctr}",
                                engine=inst.engine,
                                ins=[],
                                outs=[],
                                sync_info=mybir.SyncInfo(
                                    on_wait=extra[i : i + maxw], on_update=[]
                                ),
                            )
                        )
                    inst.sync_info = mybir.SyncInfo(
                        on_wait=keep, on_update=list(si.on_update or [])
                    )
                out.append(inst)
            if changed:
                bb.instructions = out


_BUILD_CACHE = {}


def _xt_widths(n_pad):
    """Token widths of the xt DMA tiles.  512 balances transfer time
    (728ns) against the per-DMA issue chain (~650ns SP + HWDGE): smaller
    tiles make the input head issue-bound and strictly slower (measured)."""
    widths = []
    left = n_pad
    while left > 0:
        widths.append(min(512, left))
        left -= widths[-1]
    return widths


def _build(n_chunks, ntc=0, with_bias=False, repeat=1, warm=WARM, iorder=None):
    """Build the per-core bass program.  `ntc` is unused (kept for the
    test harness call signature); `repeat` re-emits the whole body for
    marginal-cost timing; `iorder` overrides the input DMA issue order
    (list of 'x<i>'/'w<q>' tokens; any tiles not listed are appended)."""
    key = (n_chunks, with_bias, repeat, warm, tuple(iorder) if iorder else None)
    if key in _BUILD_CACHE:
        return _BUILD_CACHE[key]

    import concourse.bass as bass
    import concourse.mybir as mybir
    import concourse.tile as tile

    n_pad = n_chunks * 128
    xtw = _xt_widths(n_pad)
    n_xt = len(xtw)
    xt_off = [sum(xtw[:t]) for t in range(n_xt)]
    f8 = mybir.dt.float8e4
    bf16 = mybir.dt.bfloat16
    f32 = mybir.dt.float32
    nslots = 2 * n_chunks

    nc = bass.Bass()
    # one contiguous [128, HC, width] tensor per xt tile so each xt DMA has
    # >=1KB/partition descriptor elements (sub-512B elements cost 2x)
    xt_d = [
        nc.dram_tensor(f"xt{t}", [128, HC, w], f8, kind="ExternalInput")
        for t, w in enumerate(xtw)
    ]
    w_d = nc.dram_tensor("w", [128, HC, VS], f8, kind="ExternalInput")
    if with_bias:
        bb_d = nc.dram_tensor("bb", [1, VS], bf16, kind="ExternalInput")
    se_d = nc.dram_tensor("se", [128, nslots], f32, kind="ExternalOutput")
    # last chunk ships A_END//2 extra host columns (its ACT share is halved
    # so ACT's final accum — which gates the se DMA chain — lands earlier)
    lg_d = nc.dram_tensor(
        "lg", [n_chunks, 128, DW + A_END // 2], bf16, kind="ExternalOutput"
    )

    with tile.TileContext(nc) as tc, ExitStack() as ctx:
        consts = ctx.enter_context(tc.tile_pool(name="consts", bufs=1))
        psA = ctx.enter_context(tc.tile_pool(name="psA", bufs=2, space="PSUM"))
        psD = ctx.enter_context(tc.tile_pool(name="psD", bufs=4, space="PSUM"))
        stage = ctx.enter_context(tc.tile_pool(name="stage", bufs=3))
        outp = ctx.enter_context(tc.tile_pool(name="outp", bufs=1))

        for _rep in range(repeat):
            # ---- input DMAs, ordered so chunk0's dependencies land first
            nw = math.ceil(VS / QW)
            w_sb = []
            for q in range(nw):
                qw = min(QW, VS - q * QW)
                w_sb.append(
                    consts.tile([128, HC, qw], f8, tag=f"wq{q}", name=f"wq{q}")
                )
            xt_sb = []
            for t, w in enumerate(xtw):
                xt_sb.append(
                    consts.tile([128, HC, w], f8, tag=f"xt{t}", name=f"xt{t}")
                )

            def dma_w(q):
                nc.sync.dma_start(
                    out=w_sb[q],
                    in_=w_d[:, :, q * QW : q * QW + w_sb[q].shape[2]],
                )

            def dma_xt(t):
                nc.sync.dma_start(out=xt_sb[t], in_=xt_d[t][:, :, :])

            # DVE is the pacing engine (relay budget ~2533/chunk vs ACT's
            # 2368), so its inputs land first: xt0 + the D-region w-tiles;
            # ACT's w-tiles follow (its first exp can start ~3us later
            # without moving the finish line); later xt tiles slot into the
            # gaps just before the chunks that need them.
            dq = list(range(A_END // QW, nw))
            aq = list(range(A_END // QW))
            io = iorder or (
                ["x0"] + [f"w{q}" for q in dq] + [f"w{q}" for q in aq]
            )
            seen = set()
            for tok in io:
                i = int(tok[1:])
                if tok in seen or (tok[0] == "x" and i >= n_xt):
                    continue
                seen.add(tok)
                (dma_xt if tok[0] == "x" else dma_w)(i)
            for t in range(n_xt):
                if f"x{t}" not in seen:
                    dma_xt(t)
            for q in range(nw):
                if f"w{q}" not in seen:
                    dma_w(q)
            if with_bias:
                ones_sb = consts.tile([1, 128], bf16, tag="ones")
                nc.vector.memset(ones_sb, 1.0)
                b_sb = consts.tile([1, VS], bf16, tag="bias")
                nc.sync.dma_start(out=b_sb, in_=bb_d[0:1, :])

            se_all = outp.tile([128, nslots], f32, tag="se")
            # the last chunk only writes one accum column; zero the rest so
            # the host can sum columns uniformly
            nc.vector.memset(se_all, 0.0)

            # ---- PE warm-up: keep the tensor engine's p-state ramping while
            # the input DMAs stream.  Writes are overwritten by the first
            # real matmul (start=True); PE executes in program order.
            dummy = consts.tile([128, 128], bf16, tag="dummy")
            nc.vector.memset(dummy, 0.0)
            pw = psA.tile([128, A_END // 2], f32, name="psa")
            for _ in range(warm):
                nc.tensor.matmul(pw[:, 0:128], lhsT=dummy, rhs=dummy)

            def mm_stripe(ps, lo, hi, c):
                """Matmuls filling ps[:, 0:hi-lo] with logits for vocab
                columns [lo, hi) of 128-token chunk c."""
                tok = 128 * c
                t = max(i for i in range(n_xt) if xt_off[i] <= tok)
                xs = xt_sb[t]
                coff = tok - xt_off[t]
                cuts = [lo]
                while cuts[-1] < hi:
                    cuts.append(min((cuts[-1] // 512 + 1) * 512, hi))
                for j in range(len(cuts) - 1):
                    a, b2 = cuts[j], cuts[j + 1]
                    q, qo = a // QW, a % QW
                    pslice = ps[:, a - lo : b2 - lo]
                    for s in range(0, HC, 2):
                        nc.tensor.matmul(
                            pslice,
                            lhsT=xs[:, s : s + 2, coff : coff + 128],
                            rhs=w_sb[q][:, s : s + 2, qo : qo + (b2 - a)],
                            start=(s == 0),
                            stop=(s == HC - 2) and not with_bias,
                            perf_mode=mybir.MatmulPerfMode.DoubleRow,
                        )
                    if with_bias:
                        nc.tensor.matmul(
                            pslice,
                            lhsT=ones_sb,
                            rhs=b_sb[:, a:b2],
                            start=False,
                            stop=True,
                        )

            AS = A_END // 2  # ACT stripe width

            def emit_A(c, a_hi):
                for i in range(a_hi // AS):  # ACT stripes cover [0, a_hi)
                    ps = psA.tile([128, AS], f32, name="psa")
                    mm_stripe(ps, i * AS, (i + 1) * AS, c)
                    nc.scalar.activation(
                        out=ps,
                        in_=ps,
                        func=mybir.ActivationFunctionType.Exp,
                        scale=EXP_SCALE,
                        accum_out=se_all[:, 2 * c + i : 2 * c + i + 1],
                    )

            def emit_D(c, a_hi, last):
                # host sub-stripes: columns [a_hi, VS) in 512-col pieces
                stw = VS - a_hi
                st = stage.tile([128, stw], bf16, name=f"st{a_hi}")
                lo = a_hi
                while lo < VS:
                    hi = min(lo + 512, VS)
                    w0 = hi - lo
                    ps = psD.tile([128, 512], f32, name="psd")
                    mm_stripe(ps, lo, hi, c)
                    nc.vector.tensor_copy(
                        st[:, lo - a_hi : hi - a_hi], ps[:, :w0]
                    )
                    if last:  # split the final chunk's DMA, shortens the tail
                        nc.sync.dma_start(
                            out=lg_d[c][:, lo - a_hi : hi - a_hi],
                            in_=st[:, lo - a_hi : hi - a_hi],
                        )
                    lo = hi
                if not last:
                    nc.sync.dma_start(out=lg_d[c][:, :stw], in_=st)

            for c in range(n_chunks):
                last = c == n_chunks - 1
                if last:
                    # final chunk: halve ACT's share (its last accum gates
                    # the se DMA chain; DVE finishes earlier) and emit the
                    # ACT stripe first
                    emit_A(c, AS)
                    emit_D(c, AS, last)
                else:
                    # D sub-stripes first: keeps the PE feeding the pacing
                    # DVE as early as possible each chunk
                    emit_D(c, A_END, last)
                    emit_A(c, A_END)

            nc.sync.dma_start(out=se_d[:, :], in_=se_all)

    _split_sync_waits(nc)
    _BUILD_CACHE[key] = nc
    return nc


def _prep_inputs(output, W, b, target, tgt_idx, fp8=True):
    """Host-side sharding/layout prep. Returns (in_maps, meta)."""
    x = np.asarray(output, np.float32).reshape(PH * TL, H)
    tgt = np.asarray(target, np.int32).reshape(-1)
    ti = np.asarray(tgt_idx, np.int32)
    bv = np.asarray(b, np.float32).reshape(-1)
    with_bias = bool(np.any(bv != 0.0))

    pos = np.arange(TL)
    span = (pos[None, :] >= ti[:, :1]) & (pos[None, :] <= ti[:, 1:2])
    act = np.flatnonzero(span.reshape(-1))
    n_act = int(act.size)
    n_chunks = max(1, math.ceil(n_act / 128))
    n_pad = n_chunks * 128
    act_pad = np.zeros(n_pad, np.int64)
    act_pad[:n_act] = act

    Wf = np.asarray(W, np.float32)
    xa = x[act_pad]  # [n_pad, H] f32

    x_m = (xa * XSCALE).astype(ml_dtypes.float8_e4m3)
    w_m = (Wf * WSCALE).astype(ml_dtypes.float8_e4m3)

    # per-tile contiguous [128, HC, width] blocks (matching _build's xt_d)
    xts = {}
    off = 0
    for t, w in enumerate(_xt_widths(n_pad)):
        blk = x_m[off : off + w]  # [w, H]
        xts[f"xt{t}"] = np.ascontiguousarray(
            blk.T.reshape(HC, 128, w).transpose(1, 0, 2)
        )
        off += w

    in_maps = []
    for i in range(NCORES):
        wsh = np.ascontiguousarray(
            w_m[:, i * VS : (i + 1) * VS].reshape(HC, 128, VS).transpose(1, 0, 2)
        )
        m = {"w": wsh, **xts}
        if with_bias:
            m["bb"] = (bv[i * VS : (i + 1) * VS] * (XSCALE * WSCALE)).astype(
                ml_dtypes.bfloat16
            ).reshape(1, VS)
        in_maps.append(m)

    meta = dict(
        act=act, act_pad=act_pad, n_act=n_act, n_chunks=n_chunks, n_pad=n_pad,
        ntc=0, tgt=tgt, with_bias=with_bias, bv=bv, xa=xa, Wf=Wf,
    )
    return in_maps, meta


def _combine(results, meta):
    """Combine device sum-exp partials with host-side exp of the shipped
    bf16 logit stripes; return psk[PH, TL] (float64)."""
    n_act, n_pad, n_chunks = meta["n_act"], meta["n_pad"], meta["n_chunks"]
    total = np.zeros(n_pad, np.float64)
    for r in results:
        se = r["se"].astype(np.float64)  # [128, 2*n_chunks]
        dev = se[:, 0::2] + se[:, 1::2]  # [128, n_chunks]
        total += dev.T.reshape(-1)
        lg = r["lg"]  # [n_chunks, 128, DW + A_END//2] bf16 (raw scaled logits)
        for c in range(n_chunks):
            w0 = DW + A_END // 2 if c == n_chunks - 1 else DW
            e = np.exp(lg[c][:, :w0].astype(np.float32) * np.float32(EXP_SCALE))
            total[c * 128 : (c + 1) * 128] += e.sum(axis=1, dtype=np.float64)

    # target logits on the host (exact f64 dots; ~1500 x 512)
    xa = meta["xa"][:n_act].astype(np.float64)
    wt = meta["Wf"][:, meta["tgt"][meta["act"]]].T.astype(np.float64)
    tl = (xa * wt).sum(axis=1)
    if meta["with_bias"]:
        tl = tl + meta["bv"][meta["tgt"][meta["act"]]]

    logz = np.log(total[:n_act])
    psk = np.zeros(PH * TL, np.float64)
    psk[meta["act"]] = tl - logz
    return psk.reshape(PH, TL)


def _hmm_tail(psk, tgt_idx, states, init_logps, trans_logps, ext_logps, hsmm_sid):
    """Direct numpy port of the reference below the log-softmax."""
    ti = np.asarray(tgt_idx, np.int32)
    st4 = np.asarray(states, np.int64)
    init_logps = np.asarray(init_logps, np.float64)
    trans_logps = np.asarray(trans_logps, np.float64)
    ext_logps = np.asarray(ext_logps, np.float64)
    sid = int(np.asarray(hsmm_sid))

    pos = np.arange(TL)
    span = (pos[None, :] >= ti[:, :1]) & (pos[None, :] <= ti[:, 1:2])
    fwd_obs = np.where(span, psk, 0.0).sum(axis=1)  # [PH]

    st = st4.reshape(PH, LS)
    chain = trans_logps[st[:, :-1], st[:, 1:]].sum(axis=1)  # [PH]
    init_pmt = (init_logps[st[:, 0]] + chain).reshape(B, T, K)
    pmt = chain.reshape(B, T, K)
    obs = fwd_obs.reshape(B, T, K)
    z = np.where((np.arange(T) == 0)[None, :, None], init_pmt, pmt)
    s_first = st4[..., 0]  # [B,T,K]
    s_last = st4[..., -1]
    ov = np.any(
        st4[:, :-1, :, None, :, None] == st4[:, 1:, None, :, None, :], axis=(-1, -2)
    )  # [B,T-1,K,K]

    def lse2(x):  # logsumexp over last axis, -inf safe
        m = np.max(x, axis=-1, keepdims=True)
        ms = np.where(np.isfinite(m), m, 0.0)
        with np.errstate(divide="ignore"):
            return np.log(np.exp(x - ms).sum(axis=-1)) + ms[..., 0]

    beta = np.zeros((B, K), np.float64)
    for t in range(T - 2, -1, -1):
        sl = s_last[:, t]
        sf = s_first[:, t + 1]
        tr = (
            trans_logps[sl[:, :, None], sf[:, None, :]]
            + ext_logps[sl[:, :, None], sf[:, None, :]]
        )
        score = (
            beta[:, None, :]
            + obs[:, t + 1][:, None, :]
            + z[:, t + 1][:, None, :]
            + z[:, t][:, :, None]
            + tr
        )
        if K > 1:
            score = np.where(ov[:, t], -np.inf, score)
        beta = lse2(score)

    score0 = beta + obs[:, 0] + z[:, 0] + ext_logps[sid, s_first[:, 0]]
    log_marg = lse2(score0)
    return -np.sum(log_marg)


def kernel(output, W, b, target, tgt_idx, states, init_logps, trans_logps,
           ext_logps, hsmm_sid):
    from concourse.bass_utils import run_bass_kernel_spmd

    in_maps, meta = _prep_inputs(output, W, b, target, tgt_idx)
    nc = _build(meta["n_chunks"], meta["ntc"], meta["with_bias"])
    last_err = None
    for _attempt in range(3):
        try:
            res = run_bass_kernel_spmd(nc, in_maps, core_ids=list(range(NCORES)))
            break
        except Exception as e:  # rare transient device-unrecoverable flakes
            last_err = e
            import time as _time

            _time.sleep(2.0)
    else:
        raise last_err
    psk = _combine(res.results, meta)
    loss = _hmm_tail(psk, tgt_idx, states, init_logps, trans_logps, ext_logps, hsmm_sid)
    return np.float32(loss)
